# revision 13
# baseline (speedup 1.0000x reference)
"""BlockCirculantLinear kernel for 8x TRN2 NeuronCores.

Math: the reference's per-block circular correlation via FFT is exactly a
dense matmul out = (x * D) @ M where M[j*b+s, o*b+t] = W[o, j, (s-t) mod b].
D is folded into x on the host. The circulant blocks of M are never
materialized in DRAM: each on-chip M tile is fetched with an overlapping
-window DMA access pattern over wd = concat(W, W, axis=-1) ("window trick"):
with reversed tile columns t' = b-1-t,  M_block[s, t] = wd[o, j, 1 + s + t'],
so every SBUF row is a contiguous 512 B slice of wd. The column reversal is
undone on the host for free.

Batch is sharded across the 8 cores (data parallel, weights replicated).

Matmul dtype is float32r: fp32 storage, the PE truncates operands to the top
12 significand bits and streams at full rate (4x faster than fp32 mode, and
exact for operands with <=11 explicit mantissa bits). Measured end-to-end
relative error ~1.4e-4 (vs 2.6e-3 for bf16, 4e-7 for plain fp32 at 3.4x the
runtime). HW exec time ~531 us per core (TensorE active 477 us, 82% MFU).

Per-core device program (SPMD, same NEFF on all 8 cores):
  inputs : xT [4096, 1024] f32 ((x*D) shard, transposed on host; K on rows)
           wd [32, 32, 256] f32 (doubled W rows)
  output : outT [4096, 1024] f32 (out shard, transposed, block-reversed)

  x is cached fully in SBUF (16 MB, 8 tiles, ACT HWDGE ring). M tiles stream
  through SBUF in [128, 8, 128] window chunks (SP HWDGE ring) so the first
  matmul starts ~15 us in. For each output block-row nt, psum[t'(128),
  m-chunk(512)] accumulates over the 32 k-tiles with lhsT = M-tile
  (stationary), rhs = x-tile (moving); 4 block-row groups are in flight
  across the 8 PSUM banks.
"""

import numpy as np

B_TOTAL = 8192
D_IN = 4096
D_OUT = 4096
BLK = 128
K_IN = D_IN // BLK    # 32
K_OUT = D_OUT // BLK  # 32
N_CORES = 8
B_SHARD = B_TOTAL // N_CORES  # 1024

P = 128
KO = D_IN // P                 # 32 k-tiles of 128
XC_SPLIT = 8                   # x-cache tiles (KO/XC_SPLIT k-tiles each)
KO_PER_XC = KO // XC_SPLIT
N_TILES = K_OUT                # 32 chunks of 128 output columns
MM_FREE = 512                  # moving free dim per matmul (one PSUM bank)
M_CHUNKS = B_SHARD // MM_FREE  # 2
WDL = 2 * BLK                  # doubled-W row length
MT_CHUNKS = 4                  # window-DMA chunks per M tile
KO_PER_MT = KO // MT_CHUNKS

_compiled = None


def _wd_window_ap(bass_mod, wd, nt):
    """Overlapping-window source AP into wd [K_OUT, K_IN, WDL] for output
    block-row nt: shape [128(s), K_IN(j), 128(t')], elem = wd[nt, j, 1+s+t']."""
    return bass_mod.AP(wd, (nt * K_IN) * WDL + 1, [[1, P], [WDL, K_IN], [1, BLK]])


def _build_module():
    import concourse.bass as bass
    import concourse.tile as tile
    from concourse import bacc, mybir

    nc = bacc.Bacc("TRN2", target_bir_lowering=False, debug=False)

    f32r = mybir.dt.float32r
    f32 = mybir.dt.float32

    xT = nc.dram_tensor("xT", [D_IN, B_SHARD], f32r, kind="ExternalInput")
    wd = nc.dram_tensor("wd", [K_OUT, K_IN, WDL], f32r, kind="ExternalInput")
    outT = nc.dram_tensor("outT", [D_OUT, B_SHARD], f32, kind="ExternalOutput")

    xT_v = xT.rearrange("(ko p) m -> p ko m", p=P)

    with tile.TileContext(nc) as tc:
        with (
            tc.tile_pool(name="xcache", bufs=1) as xpool,
            tc.tile_pool(name="mtiles", bufs=16) as mpool,
            tc.tile_pool(name="otiles", bufs=3) as opool,
            tc.tile_pool(name="psum", bufs=4, space="PSUM") as psum_pool,
        ):
            # x caches go on the ACT HWDGE ring; M-tile window loads use the
            # SP HWDGE ring — two parallel FIFOs, so neither queues behind
            # the other and the first matmul can start ~15 us in
            xcs = []
            for xi in range(XC_SPLIT):
                xc = xpool.tile([P, KO_PER_XC, B_SHARD], f32r, name=f"xc{xi}")
                nc.scalar.dma_start(
                    xc[:], xT_v[:, xi * KO_PER_XC : (xi + 1) * KO_PER_XC, :]
                )
                xcs.append(xc)

            def load_mt_chunk(nt, mi):
                mt = mpool.tile(
                    [P, KO_PER_MT, BLK], f32r, tag="mt", name=f"mt_{nt}_{mi}"
                )
                src = _wd_window_ap(bass, wd, nt)
                nc.sync.dma_start(
                    mt[:], src[:, mi * KO_PER_MT : (mi + 1) * KO_PER_MT, :]
                )
                return mt

            def alloc_psums(nt):
                return [
                    psum_pool.tile([P, MM_FREE], f32, tag=f"ps{i}", name=f"ps{i}_{nt}")
                    for i in range(M_CHUNKS)
                ]

            def evict(nt, psums):
                ot = opool.tile([P, B_SHARD], f32, tag="ot", name=f"ot{nt}")
                for mc in range(M_CHUNKS):
                    nc.vector.tensor_copy(
                        ot[:, mc * MM_FREE : (mc + 1) * MM_FREE], psums[mc][:]
                    )
                nc.sync.dma_start(outT[nt * BLK : (nt + 1) * BLK, :], ot[:])

            # Phase 1: while x is still streaming in, run the first PHASE1
            # block-rows ko-major-interleaved across all 8 PSUM banks, so
            # each arriving x chunk feeds PHASE1 accumulation groups and PE
            # consumption matches the ~300 GB/s x supply instead of
            # stalling. M-tile chunks are issued mi-major to match.
            PHASE1 = 4
            p1_mts = {}
            for mi in range(MT_CHUNKS):
                for nt in range(PHASE1):
                    p1_mts[(nt, mi)] = load_mt_chunk(nt, mi)
            p1_psums = {nt: alloc_psums(nt) for nt in range(PHASE1)}
            for ko in range(KO):
                xc = xcs[ko // KO_PER_XC]
                kk = ko % KO_PER_XC
                for nt in range(PHASE1):
                    mt = p1_mts[(nt, ko // KO_PER_MT)]
                    for mc in range(M_CHUNKS):
                        nc.tensor.matmul(
                            p1_psums[nt][mc][:],
                            lhsT=mt[:, ko % KO_PER_MT, :],
                            rhs=xc[:, kk, mc * MM_FREE : (mc + 1) * MM_FREE],
                            start=(ko == 0),
                            stop=(ko == KO - 1),
                        )
            for nt in range(PHASE1):
                evict(nt, p1_psums[nt])

            # Phase 2: x fully resident; depth-first per block-row (ko-major
            # across many groups would starve on M-window DMA descriptors)
            for nt in range(PHASE1, N_TILES):
                mts = [load_mt_chunk(nt, mi) for mi in range(MT_CHUNKS)]
                psums = alloc_psums(nt)
                for ko in range(KO):
                    xc = xcs[ko // KO_PER_XC]
                    kk = ko % KO_PER_XC
                    mt = mts[ko // KO_PER_MT]
                    for mc in range(M_CHUNKS):
                        nc.tensor.matmul(
                            psums[mc][:],
                            lhsT=mt[:, ko % KO_PER_MT, :],
                            rhs=xc[:, kk, mc * MM_FREE : (mc + 1) * MM_FREE],
                            start=(ko == 0),
                            stop=(ko == KO - 1),
                        )
                evict(nt, psums)

    nc.compile()
    return nc


def _get_module():
    global _compiled
    if _compiled is None:
        _compiled = _build_module()
    return _compiled


def kernel(x: np.ndarray, W: np.ndarray, D_bernoulli: np.ndarray) -> np.ndarray:
    from concourse.bass_utils import run_bass_kernel_spmd

    x = np.asarray(x, dtype=np.float32)
    W = np.asarray(W, dtype=np.float32)
    D = np.asarray(D_bernoulli, dtype=np.float32)

    xd = x * D[None, :]
    wd = np.ascontiguousarray(np.concatenate([W, W], axis=-1))  # [32, 32, 256]

    in_maps = []
    for c in range(N_CORES):
        xs = np.ascontiguousarray(xd[c * B_SHARD : (c + 1) * B_SHARD].T)
        in_maps.append({"xT": xs, "wd": wd})

    nc = _get_module()
    res = run_bass_kernel_spmd(nc, in_maps, core_ids=list(range(N_CORES)))

    out = np.empty((B_TOTAL, D_OUT), dtype=np.float32)
    for c in range(N_CORES):
        oT = res.results[c]["outT"]                      # [4096, 1024]
        oT = oT.reshape(K_OUT, BLK, B_SHARD)[:, ::-1, :] # undo column reversal
        out[c * B_SHARD : (c + 1) * B_SHARD] = oT.reshape(D_OUT, B_SHARD).T
    return out


# revision 14
# speedup vs baseline: 1.0148x; 1.0148x over previous
"""BlockCirculantLinear kernel for 8x TRN2 NeuronCores.

Math: the reference's per-block circular correlation via FFT is exactly a
dense matmul out = (x * D) @ M where M[j*b+s, o*b+t] = W[o, j, (s-t) mod b].
D is folded into x on the host. The circulant blocks of M are never
materialized in DRAM: each on-chip M tile is fetched with an overlapping
-window DMA access pattern over wd = concat(W, W, axis=-1) ("window trick"):
with reversed tile columns t' = b-1-t,  M_block[s, t] = wd[o, j, 1 + s + t'],
so every SBUF row is a contiguous 512 B slice of wd. The column reversal is
undone on the host for free.

Batch is sharded across the 8 cores (data parallel, weights replicated).

Matmul dtype is float32r: fp32 storage, the PE truncates operands to the top
12 significand bits and streams at full rate (4x faster than fp32 mode, and
exact for operands with <=11 explicit mantissa bits). Measured end-to-end
relative error ~1.4e-4 (vs 2.6e-3 for bf16, 4e-7 for plain fp32 at 3.4x the
runtime). HW exec time ~531 us per core (TensorE active 477 us, 82% MFU).

Per-core device program (SPMD, same NEFF on all 8 cores):
  inputs : xT [4096, 1024] f32 ((x*D) shard, transposed on host; K on rows)
           wd [32, 32, 256] f32 (doubled W rows)
  output : outT [4096, 1024] f32 (out shard, transposed, block-reversed)

  x is cached fully in SBUF (16 MB, 8 tiles, ACT HWDGE ring). M tiles stream
  through SBUF in [128, 8, 128] window chunks (SP HWDGE ring) so the first
  matmul starts ~15 us in. For each output block-row nt, psum[t'(128),
  m-chunk(512)] accumulates over the 32 k-tiles with lhsT = M-tile
  (stationary), rhs = x-tile (moving); 4 block-row groups are in flight
  across the 8 PSUM banks.
"""

import numpy as np

B_TOTAL = 8192
D_IN = 4096
D_OUT = 4096
BLK = 128
K_IN = D_IN // BLK    # 32
K_OUT = D_OUT // BLK  # 32
N_CORES = 8
B_SHARD = B_TOTAL // N_CORES  # 1024

P = 128
KO = D_IN // P                 # 32 k-tiles of 128
XC_SPLIT = 8                   # x-cache tiles (KO/XC_SPLIT k-tiles each)
KO_PER_XC = KO // XC_SPLIT
N_TILES = K_OUT                # 32 chunks of 128 output columns
MM_FREE = 512                  # moving free dim per matmul (one PSUM bank)
M_CHUNKS = B_SHARD // MM_FREE  # 2
WDL = 2 * BLK                  # doubled-W row length
MT_CHUNKS = 4                  # window-DMA chunks per M tile
KO_PER_MT = KO // MT_CHUNKS

_compiled = None


def _wd_window_ap(bass_mod, wd, nt):
    """Overlapping-window source AP into wd [K_OUT, K_IN, WDL] for output
    block-row nt: shape [128(s), K_IN(j), 128(t')], elem = wd[nt, j, 1+s+t']."""
    return bass_mod.AP(wd, (nt * K_IN) * WDL + 1, [[1, P], [WDL, K_IN], [1, BLK]])


def _build_module():
    import concourse.bass as bass
    import concourse.tile as tile
    from concourse import bacc, mybir

    nc = bacc.Bacc("TRN2", target_bir_lowering=False, debug=False)

    f32r = mybir.dt.float32r
    f32 = mybir.dt.float32

    xT = nc.dram_tensor("xT", [D_IN, B_SHARD], f32r, kind="ExternalInput")
    wd = nc.dram_tensor("wd", [K_OUT, K_IN, WDL], f32r, kind="ExternalInput")
    outT = nc.dram_tensor("outT", [D_OUT, B_SHARD], f32, kind="ExternalOutput")

    xT_v = xT.rearrange("(ko p) m -> p ko m", p=P)

    with tile.TileContext(nc) as tc:
        with (
            tc.tile_pool(name="xcache", bufs=1) as xpool,
            tc.tile_pool(name="mtiles", bufs=12) as mpool,
            tc.tile_pool(name="otiles", bufs=3) as opool,
            tc.tile_pool(name="psum", bufs=4, space="PSUM") as psum_pool,
        ):
            # x caches go on the ACT HWDGE ring; M-tile window loads use the
            # SP HWDGE ring — two parallel FIFOs, so neither queues behind
            # the other and the first matmul can start ~15 us in
            xcs = []
            for xi in range(XC_SPLIT):
                xc = xpool.tile([P, KO_PER_XC, B_SHARD], f32r, name=f"xc{xi}")
                nc.scalar.dma_start(
                    xc[:], xT_v[:, xi * KO_PER_XC : (xi + 1) * KO_PER_XC, :]
                )
                xcs.append(xc)

            for nt in range(N_TILES):
                mts = []
                for mi in range(MT_CHUNKS):
                    mt = mpool.tile(
                        [P, KO_PER_MT, BLK], f32r, tag="mt", name=f"mt_{nt}_{mi}"
                    )
                    src = _wd_window_ap(bass, wd, nt)
                    nc.sync.dma_start(
                        mt[:], src[:, mi * KO_PER_MT : (mi + 1) * KO_PER_MT, :]
                    )
                    mts.append(mt)
                psums = [
                    psum_pool.tile([P, MM_FREE], f32, tag=f"ps{i}", name=f"ps{i}_{nt}")
                    for i in range(M_CHUNKS)
                ]
                for ko in range(KO):
                    xc = xcs[ko // KO_PER_XC]
                    kk = ko % KO_PER_XC
                    mt = mts[ko // KO_PER_MT]
                    for mc in range(M_CHUNKS):
                        nc.tensor.matmul(
                            psums[mc][:],
                            lhsT=mt[:, ko % KO_PER_MT, :],
                            rhs=xc[:, kk, mc * MM_FREE : (mc + 1) * MM_FREE],
                            start=(ko == 0),
                            stop=(ko == KO - 1),
                        )
                ot = opool.tile([P, B_SHARD], f32, tag="ot", name=f"ot{nt}")
                for mc in range(M_CHUNKS):
                    nc.vector.tensor_copy(
                        ot[:, mc * MM_FREE : (mc + 1) * MM_FREE], psums[mc][:]
                    )
                nc.sync.dma_start(outT[nt * BLK : (nt + 1) * BLK, :], ot[:])

    nc.compile()
    return nc


def _get_module():
    global _compiled
    if _compiled is None:
        _compiled = _build_module()
    return _compiled


def kernel(x: np.ndarray, W: np.ndarray, D_bernoulli: np.ndarray) -> np.ndarray:
    from concourse.bass_utils import run_bass_kernel_spmd

    x = np.asarray(x, dtype=np.float32)
    W = np.asarray(W, dtype=np.float32)
    D = np.asarray(D_bernoulli, dtype=np.float32)

    xd = x * D[None, :]
    wd = np.ascontiguousarray(np.concatenate([W, W], axis=-1))  # [32, 32, 256]

    in_maps = []
    for c in range(N_CORES):
        xs = np.ascontiguousarray(xd[c * B_SHARD : (c + 1) * B_SHARD].T)
        in_maps.append({"xT": xs, "wd": wd})

    nc = _get_module()
    res = run_bass_kernel_spmd(nc, in_maps, core_ids=list(range(N_CORES)))

    out = np.empty((B_TOTAL, D_OUT), dtype=np.float32)
    for c in range(N_CORES):
        oT = res.results[c]["outT"]                      # [4096, 1024]
        oT = oT.reshape(K_OUT, BLK, B_SHARD)[:, ::-1, :] # undo column reversal
        out[c * B_SHARD : (c + 1) * B_SHARD] = oT.reshape(D_OUT, B_SHARD).T
    return out


# revision 15
# speedup vs baseline: 1.0208x; 1.0059x over previous
"""BlockCirculantLinear kernel for 8x TRN2 NeuronCores.

Math: the reference's per-block circular correlation via FFT is exactly a
dense matmul out = (x * D) @ M where M[j*b+s, o*b+t] = W[o, j, (s-t) mod b].
D is folded into x on the host. The circulant blocks of M are never
materialized in DRAM: each on-chip M tile is fetched with an overlapping
-window DMA access pattern over wd = concat(W, W, axis=-1) ("window trick"):
with reversed tile columns t' = b-1-t,  M_block[s, t] = wd[o, j, 1 + s + t'],
so every SBUF row is a contiguous 512 B slice of wd. The column reversal is
undone on the host for free.

Batch is sharded across the 8 cores (data parallel, weights replicated).

Matmul dtype is float32r: fp32 storage, the PE truncates operands to the top
12 significand bits and streams at full rate (4x faster than fp32 mode, and
exact for operands with <=11 explicit mantissa bits). Measured end-to-end
relative error ~1.4e-4 (vs 2.6e-3 for bf16, 4e-7 for plain fp32 at 3.4x the
runtime). HW exec time ~531 us per core (TensorE active 477 us, 82% MFU).

Per-core device program (SPMD, same NEFF on all 8 cores):
  inputs : xT [128, 32, 1024] f32 ((x*D) shard, partition-major tiled)
           wd [32, 32, 256] f32 (doubled W rows)
  output : outT [4096, 1024] f32 (out shard, transposed, block-reversed)

  x is cached fully in SBUF (16 MB, 16 tiles, ACT HWDGE ring). M tiles stream
  through SBUF in [128, 8, 128] window chunks (SP HWDGE ring) so the first
  matmul starts ~15 us in. For each output block-row nt, psum[t'(128),
  m-chunk(512)] accumulates over the 32 k-tiles with lhsT = M-tile
  (stationary), rhs = x-tile (moving); 4 block-row groups are in flight
  across the 8 PSUM banks.
"""

import numpy as np

B_TOTAL = 8192
D_IN = 4096
D_OUT = 4096
BLK = 128
K_IN = D_IN // BLK    # 32
K_OUT = D_OUT // BLK  # 32
N_CORES = 8
B_SHARD = B_TOTAL // N_CORES  # 1024

P = 128
KO = D_IN // P                 # 32 k-tiles of 128
XC_SPLIT = 16                  # x-cache tiles (KO/XC_SPLIT k-tiles each)
KO_PER_XC = KO // XC_SPLIT
N_TILES = K_OUT                # 32 chunks of 128 output columns
MM_FREE = 512                  # moving free dim per matmul (one PSUM bank)
M_CHUNKS = B_SHARD // MM_FREE  # 2
WDL = 2 * BLK                  # doubled-W row length
MT_CHUNKS = 4                  # window-DMA chunks per M tile
KO_PER_MT = KO // MT_CHUNKS

_compiled = None


def _wd_window_ap(bass_mod, wd, nt):
    """Overlapping-window source AP into wd [K_OUT, K_IN, WDL] for output
    block-row nt: shape [128(s), K_IN(j), 128(t')], elem = wd[nt, j, 1+s+t']."""
    return bass_mod.AP(wd, (nt * K_IN) * WDL + 1, [[1, P], [WDL, K_IN], [1, BLK]])


def _build_module():
    import concourse.bass as bass
    import concourse.tile as tile
    from concourse import bacc, mybir

    nc = bacc.Bacc("TRN2", target_bir_lowering=False, debug=False)

    f32r = mybir.dt.float32r
    f32 = mybir.dt.float32

    xT = nc.dram_tensor("xT", [P, KO, B_SHARD], f32r, kind="ExternalInput")
    wd = nc.dram_tensor("wd", [K_OUT, K_IN, WDL], f32r, kind="ExternalInput")
    outT = nc.dram_tensor("outT", [D_OUT, B_SHARD], f32, kind="ExternalOutput")

    with tile.TileContext(nc) as tc:
        with (
            tc.tile_pool(name="xcache", bufs=1) as xpool,
            tc.tile_pool(name="mtiles", bufs=12) as mpool,
            tc.tile_pool(name="otiles", bufs=3) as opool,
            tc.tile_pool(name="psum", bufs=4, space="PSUM") as psum_pool,
        ):
            # x caches go on the ACT HWDGE ring; M-tile window loads use the
            # SP HWDGE ring — two parallel FIFOs, so neither queues behind
            # the other and the first matmul can start ~15 us in
            xcs = []
            for xi in range(XC_SPLIT):
                xc = xpool.tile([P, KO_PER_XC, B_SHARD], f32r, name=f"xc{xi}")
                nc.scalar.dma_start(
                    xc[:], xT[:, xi * KO_PER_XC : (xi + 1) * KO_PER_XC, :]
                )
                xcs.append(xc)

            for nt in range(N_TILES):
                mts = []
                for mi in range(MT_CHUNKS):
                    mt = mpool.tile(
                        [P, KO_PER_MT, BLK], f32r, tag="mt", name=f"mt_{nt}_{mi}"
                    )
                    src = _wd_window_ap(bass, wd, nt)
                    nc.sync.dma_start(
                        mt[:], src[:, mi * KO_PER_MT : (mi + 1) * KO_PER_MT, :]
                    )
                    mts.append(mt)
                psums = [
                    psum_pool.tile([P, MM_FREE], f32, tag=f"ps{i}", name=f"ps{i}_{nt}")
                    for i in range(M_CHUNKS)
                ]
                for ko in range(KO):
                    xc = xcs[ko // KO_PER_XC]
                    kk = ko % KO_PER_XC
                    mt = mts[ko // KO_PER_MT]
                    for mc in range(M_CHUNKS):
                        nc.tensor.matmul(
                            psums[mc][:],
                            lhsT=mt[:, ko % KO_PER_MT, :],
                            rhs=xc[:, kk, mc * MM_FREE : (mc + 1) * MM_FREE],
                            start=(ko == 0),
                            stop=(ko == KO - 1),
                        )
                ot = opool.tile([P, B_SHARD], f32, tag="ot", name=f"ot{nt}")
                for mc in range(M_CHUNKS):
                    nc.vector.tensor_copy(
                        ot[:, mc * MM_FREE : (mc + 1) * MM_FREE], psums[mc][:]
                    )
                nc.sync.dma_start(outT[nt * BLK : (nt + 1) * BLK, :], ot[:])

    nc.compile()
    return nc


def _get_module():
    global _compiled
    if _compiled is None:
        _compiled = _build_module()
    return _compiled


def kernel(x: np.ndarray, W: np.ndarray, D_bernoulli: np.ndarray) -> np.ndarray:
    from concourse.bass_utils import run_bass_kernel_spmd

    x = np.asarray(x, dtype=np.float32)
    W = np.asarray(W, dtype=np.float32)
    D = np.asarray(D_bernoulli, dtype=np.float32)

    xd = x * D[None, :]
    wd = np.ascontiguousarray(np.concatenate([W, W], axis=-1))  # [32, 32, 256]

    in_maps = []
    for c in range(N_CORES):
        xs = xd[c * B_SHARD : (c + 1) * B_SHARD].T          # [4096, 1024]
        # partition-major pre-tiling: [p, ko, m], 8KB-contiguous per p-chunk
        xs = np.ascontiguousarray(
            xs.reshape(KO, P, B_SHARD).transpose(1, 0, 2)
        )
        in_maps.append({"xT": xs, "wd": wd})

    nc = _get_module()
    res = run_bass_kernel_spmd(nc, in_maps, core_ids=list(range(N_CORES)))

    out = np.empty((B_TOTAL, D_OUT), dtype=np.float32)
    for c in range(N_CORES):
        oT = res.results[c]["outT"]                      # [4096, 1024]
        oT = oT.reshape(K_OUT, BLK, B_SHARD)[:, ::-1, :] # undo column reversal
        out[c * B_SHARD : (c + 1) * B_SHARD] = oT.reshape(D_OUT, B_SHARD).T
    return out
